# revision 5
# baseline (speedup 1.0000x reference)
"""Trainium2 Bass kernel: float32 -> 32-channel bit-plane encoding.

For input x [4096, 512] f32, produces out [4096, 512, 32] f32 where
out[b, f, 0] = (x[b,f] < 0) and out[b, f, 1+j] = bit (30-j) of
bitcast_int32(|x[b,f]|), MSB first.

Wire-format design: every output element is exactly 0.0 or 1.0, so the
device computes and stores each of the 67M output elements as a uint8
{0,1}; the host applies a value-preserving widening cast to f32.  This
cuts device HBM write traffic 4x (8MB/core instead of 32MB/core), which
is the binding roofline (per-NeuronCore HBM/fabric bandwidth ~440 GB/s
observed).

Host-side repack makes the device compute uniform:
  i' = (bitcast_u32(x) & 0x7FFFFFFF) | ((x < 0) << 31)
stored as a big-endian byte stream, viewed as uint16 pairs.  Then output
channel k of feature f equals bit (7 - k%8) of stream byte 4f + k//8.

Device compute (VectorE only, one fused op per bit-plane):
  plane_m = (x_u16 >> (7-m)) & 0x0101     m = 0..7
Each uint16 tensor_scalar element yields TWO planar output bytes, and
the dense step-1 16-bit single-src pattern hits the DVE 4x perf mode
(~4 elem/cycle), so vector busy is ~12us/core -- under the DMA shadow.

The planes are written to HBM planar (per 128-row tile: 8 planes x 2048
bytes); the host interleaves planes into the [rows, F, 32] layout during
the f32 cast.

Pipelining: input DMAs ride the scalar-engine HWDGE ring so the sync
ring carries only the 8MB output stream; row-tile 0 is processed at
finer granularity (column-split compute, per-plane output pieces) to
collapse the startup ramp.

Sharded row-wise over 8 NeuronCores (512 rows each, 4 row tiles of 128).
"""

import sys

if "/opt/trn_rl_repo" not in sys.path:
    sys.path.insert(0, "/opt/trn_rl_repo")

import numpy as np

import concourse.bass as bass
import concourse.mybir as mybir

P = 128           # SBUF partitions
F = 512           # features per row
K = 32            # output channels per feature
N_CORES = 8
ROWS_TOTAL = 4096
ROWS = ROWS_TOTAL // N_CORES   # rows per core (512)
NRT = ROWS // P                # row tiles per core (4)
W16 = F * 2                    # uint16 words per row (1024)
PLANES = 8                     # bit planes per byte
OW = PLANES * W16              # output uint16 per row (8192)


def _schedule():
    """Build (chunks, pieces, in_splits).

    chunk: (rt, plane, col_lo, col_hi, in_part)  -- one tensor_scalar op;
        needs input columns up to col_hi of row tile rt (in_part = index
        of the in-DMA split it depends on).
    piece: (rt, u16_lo, u16_hi, last_chunk_idx)  -- one out-DMA.
    in_splits[rt]: list of (col_lo, col_hi) in-DMA slices for that tile.
    """
    chunks, pieces, in_splits = [], [], []
    # rt0: input in 2 column halves; compute col-split; per-plane pieces
    in_splits.append([(0, W16 // 2), (W16 // 2, W16)])
    for m in range(PLANES):
        chunks.append((0, m, 0, W16 // 2, 0))
        chunks.append((0, m, W16 // 2, W16, 1))
        pieces.append((0, m * W16, (m + 1) * W16, len(chunks) - 1))
    # rt1-3: single input DMA, full-width planes, 2 pieces of 4 planes
    for rt in range(1, NRT):
        in_splits.append([(0, W16)])
        for m in range(PLANES):
            chunks.append((rt, m, 0, W16, 0))
            if m % 4 == 3:
                pieces.append(
                    (rt, (m - 3) * W16, (m + 1) * W16, len(chunks) - 1))
    return chunks, pieces, in_splits


def build_nc() -> bass.Bass:
    nc = bass.Bass("TRN2", target_bir_lowering=False, debug=False)
    u16 = mybir.dt.uint16

    xin = nc.declare_dram_parameter("xin", [ROWS, W16], u16, isOutput=False)
    out = nc.declare_dram_parameter("out", [ROWS, OW], u16, isOutput=True)
    xin_ap, out_ap = xin.ap(), out.ap()

    chunks, pieces, in_splits = _schedule()

    from contextlib import ExitStack
    with ExitStack() as ctx:
        xt = [ctx.enter_context(nc.sbuf_tensor(f"xt{b}", [P, W16], u16))
              for b in range(NRT)]
        ot = [ctx.enter_context(nc.sbuf_tensor(f"ot{b}", [P, OW], u16))
              for b in range(NRT)]

        in_sem = [ctx.enter_context(nc.semaphore(f"in_sem{b}"))
                  for b in range(NRT)]
        ts_sem = ctx.enter_context(nc.semaphore("ts_sem"))
        od_sem = ctx.enter_context(nc.semaphore("od_sem"))

        ctx.enter_context(nc.Block())
        block = nc.cur_block

        @block.scalar
        def _(sc: bass.BassEngine):
            # input stream on the ACT HWDGE ring, out of the way of the
            # output stream on the SP ring
            for rt in range(NRT):
                for (lo, hi) in in_splits[rt]:
                    sc.dma_start(
                        xt[rt][:, lo:hi], xin_ap[rt * P:(rt + 1) * P, lo:hi]
                    ).then_inc(in_sem[rt], 16)

        @block.vector
        def _(vec: bass.BassEngine):
            seen = {}
            for (rt, m, lo, hi, ip) in chunks:
                if seen.get(rt, -1) < ip:
                    vec.wait_ge(in_sem[rt], 16 * (ip + 1))
                    seen[rt] = ip
                vec.tensor_scalar(
                    ot[rt][:, m * W16 + lo:m * W16 + hi],
                    xt[rt][:, lo:hi],
                    7 - m,
                    0x0101,
                    mybir.AluOpType.logical_shift_right,
                    mybir.AluOpType.bitwise_and,
                ).then_inc(ts_sem)

        @block.sync
        def _(sp: bass.BassEngine):
            for (rt, lo, hi, lc) in pieces:
                sp.wait_ge(ts_sem, lc + 1)
                sp.dma_start(
                    out_ap[rt * P:(rt + 1) * P, lo:hi],
                    ot[rt][:, lo:hi],
                ).then_inc(od_sem, 16)

    return nc


_NC_CACHE = None


def _get_nc():
    global _NC_CACHE
    if _NC_CACHE is None:
        _NC_CACHE = build_nc()
    return _NC_CACHE


def pack_shard(x_shard: np.ndarray) -> np.ndarray:
    """[ROWS, F] f32 -> [ROWS, W16] uint16: sign-normalized bitcast words
    as a big-endian byte stream, viewed as little-endian uint16 pairs."""
    x_shard = np.ascontiguousarray(x_shard)
    xi = (x_shard.view(np.uint32) & np.uint32(0x7FFFFFFF)) | \
        ((x_shard < 0).astype(np.uint32) << np.uint32(31))
    return xi.byteswap().view(np.uint16)


def unpack_shard(raw: np.ndarray) -> np.ndarray:
    """[ROWS, OW] uint16 planar planes -> [ROWS, F, K] f32."""
    b = raw.view(np.uint8).reshape(ROWS, PLANES, F, 4)
    return b.transpose(0, 2, 3, 1).reshape(ROWS, F, K).astype(np.float32)


def kernel(x: np.ndarray) -> np.ndarray:
    from concourse.bass_utils import run_bass_kernel_spmd

    x = np.asarray(x, dtype=np.float32)
    assert x.shape == (ROWS_TOTAL, F), x.shape
    nc = _get_nc()
    in_maps = [
        {"xin": pack_shard(x[i * ROWS:(i + 1) * ROWS])} for i in range(N_CORES)
    ]
    res = run_bass_kernel_spmd(nc, in_maps, list(range(N_CORES)))
    parts = [unpack_shard(res.results[i]["out"]) for i in range(N_CORES)]
    return np.concatenate(parts, axis=0)


# revision 8
# speedup vs baseline: 1.0010x; 1.0010x over previous
"""Trainium2 Bass kernel: float32 -> 32-channel bit-plane encoding.

For input x [4096, 512] f32, produces out [4096, 512, 32] f32 where
out[b, f, 0] = (x[b,f] < 0) and out[b, f, 1+j] = bit (30-j) of
bitcast_int32(|x[b,f]|), MSB first.

Wire-format design: every output element is exactly 0.0 or 1.0, so the
device computes and stores each of the 67M output elements as a uint8
{0,1}; the host applies a value-preserving widening cast to f32.  This
cuts device HBM write traffic 4x (8MB/core instead of 32MB/core), which
is the binding roofline (per-NeuronCore HBM/fabric bandwidth ~430 GB/s
observed).

Host-side repack makes the device compute uniform:
  i' = (bitcast_u32(x) & 0x7FFFFFFF) | ((x < 0) << 31)
stored as a big-endian byte stream, viewed as uint16 pairs.  Then output
channel k of feature f equals bit (7 - k%8) of stream byte 4f + k//8.

Device compute (VectorE), one fused tensor_scalar op per bit plane:
  plane_m = (x_u16 >> (7-m)) & 0x0101     m = 0..7
Each uint16 element yields TWO planar output bytes; the dense step-1
16-bit single-src pattern hits the DVE 4x perf mode (~4 elem/cycle).
Row tile 0 runs as 8 narrow ops (starts as soon as the first 256KB
input lands); row tiles 1-3 are fused into 8 wide FD=3072 ops to
amortize per-op overhead; the last plane is re-split 3x so the final
output pieces can issue early.  Vector busy ~9us, under the DMA shadow.

Output-DMA issue (HWDGE descriptor generation, ~1.3us/MB of sequencer
time) is split across both HWDGE rings: SyncE and ScalarE each issue
half the pieces; ScalarE also issues the input loads first.

The planes land in HBM planar per SBUF partition; the host interleaves
planes/tiles into [rows, F, 32] during the f32 cast.

Sharded row-wise over 8 NeuronCores (512 rows each, 4 row tiles of 128).
"""

import sys

if "/opt/trn_rl_repo" not in sys.path:
    sys.path.insert(0, "/opt/trn_rl_repo")

import numpy as np

import concourse.bass as bass
import concourse.mybir as mybir

P = 128           # SBUF partitions
F = 512           # features per row
K = 32            # output channels per feature
N_CORES = 8
ROWS_TOTAL = 4096
ROWS = ROWS_TOTAL // N_CORES   # rows per core (512)
NRT = ROWS // P                # row tiles per core (4)
W16 = F * 2                    # uint16 words per row (1024)
PLANES = 8                     # bit planes per byte
MW = (NRT - 1) * W16           # merged row-tile width (3072 u16)
OW0 = PLANES * W16             # rt0 output section (8192 u16/partition)
OWM = PLANES * MW              # merged output section (24576 u16/partition)
OCOLS = OW0 + OWM              # output dram columns per partition (32768)


def build_nc() -> bass.Bass:
    nc = bass.Bass("TRN2", target_bir_lowering=False, debug=False)
    u16 = mybir.dt.uint16

    xin = nc.declare_dram_parameter("xin", [ROWS, W16], u16, isOutput=False)
    out = nc.declare_dram_parameter("out", [P, OCOLS], u16, isOutput=True)
    xin_ap, out_ap = xin.ap(), out.ap()

    shift_and = (mybir.AluOpType.logical_shift_right,
                 mybir.AluOpType.bitwise_and)

    from contextlib import ExitStack
    with ExitStack() as ctx:
        xt0 = ctx.enter_context(nc.sbuf_tensor("xt0", [P, W16], u16))
        xtm = ctx.enter_context(nc.sbuf_tensor("xtm", [P, MW], u16))
        ot0 = ctx.enter_context(nc.sbuf_tensor("ot0", [P, OW0], u16))
        otm = ctx.enter_context(nc.sbuf_tensor("otm", [P, OWM], u16))

        in_sem = [ctx.enter_context(nc.semaphore(f"in_sem{b}"))
                  for b in range(NRT)]
        ts_sem = ctx.enter_context(nc.semaphore("ts_sem"))
        od_sem = ctx.enter_context(nc.semaphore("od_sem"))

        ctx.enter_context(nc.Block())
        block = nc.cur_block

        # vector op schedule: (kind, m, lo, hi) with kind 0 = rt0 section
        # (xt0/ot0), 1 = merged section (xtm/otm); [lo,hi) in u16 columns
        # within the plane.  Last merged plane split 3x for early pieces.
        vops = [(0, m, 0, W16) for m in range(PLANES)]
        vops += [(1, m, 0, MW) for m in range(PLANES - 1)]
        vops += [(1, PLANES - 1, j * W16, (j + 1) * W16)
                 for j in range(NRT - 1)]
        # out pieces: (engine_idx, col_lo, col_hi, ts_count) in dram cols
        pieces = [
            (0, 0, 4 * W16, 4),                    # rt0 planes 0-3 (1MB)
            (0, 4 * W16, 8 * W16, 8),              # rt0 planes 4-7 (1MB)
        ]
        for m in range(PLANES - 1):                # merged planes (0.75MB)
            pieces.append((m % 2, OW0 + m * MW, OW0 + (m + 1) * MW, 9 + m))
        base = OW0 + (PLANES - 1) * MW
        for j in range(NRT - 1):                   # last plane 3x 0.25MB
            pieces.append((1, base + j * W16, base + (j + 1) * W16, 16 + j))

        @block.vector
        def _(vec: bass.BassEngine):
            waited_m = False
            for (kind, m, lo, hi) in vops:
                if kind == 0 and m == 0:
                    vec.wait_ge(in_sem[0], 16)
                elif kind == 1 and not waited_m:
                    for rt in range(1, NRT):
                        vec.wait_ge(in_sem[rt], 16)
                    waited_m = True
                xt, ot, w = (xt0, ot0, W16) if kind == 0 else (xtm, otm, MW)
                vec.tensor_scalar(
                    ot[:, m * w + lo:m * w + hi],
                    xt[:, lo:hi],
                    7 - m,
                    0x0101,
                    *shift_and,
                ).then_inc(ts_sem)

        @block.scalar
        def _(sc: bass.BassEngine):
            sc.dma_start(xt0[:], xin_ap[0:P, :]).then_inc(in_sem[0], 16)
            for rt in range(1, NRT):
                sc.dma_start(
                    xtm[:, (rt - 1) * W16:rt * W16],
                    xin_ap[rt * P:(rt + 1) * P, :],
                ).then_inc(in_sem[rt], 16)
            for (eng, lo, hi, n) in pieces:
                if eng != 1:
                    continue
                sc.wait_ge(ts_sem, n)
                sc.dma_start(out_ap[:, lo:hi], ot0[:, lo:hi] if hi <= OW0
                             else otm[:, lo - OW0:hi - OW0]
                             ).then_inc(od_sem, 16)

        @block.sync
        def _(sp: bass.BassEngine):
            for (eng, lo, hi, n) in pieces:
                if eng != 0:
                    continue
                sp.wait_ge(ts_sem, n)
                sp.dma_start(out_ap[:, lo:hi], ot0[:, lo:hi] if hi <= OW0
                             else otm[:, lo - OW0:hi - OW0]
                             ).then_inc(od_sem, 16)

    return nc


_NC_CACHE = None


def _get_nc():
    global _NC_CACHE
    if _NC_CACHE is None:
        _NC_CACHE = build_nc()
    return _NC_CACHE


def pack_shard(x_shard: np.ndarray) -> np.ndarray:
    """[ROWS, F] f32 -> [ROWS, W16] uint16: sign-normalized bitcast words
    as a big-endian byte stream, viewed as little-endian uint16 pairs."""
    x_shard = np.ascontiguousarray(x_shard)
    xi = (x_shard.view(np.uint32) & np.uint32(0x7FFFFFFF)) | \
        ((x_shard < 0).astype(np.uint32) << np.uint32(31))
    return xi.byteswap().view(np.uint16)


def unpack_shard(raw: np.ndarray) -> np.ndarray:
    """[P, OCOLS] uint16 planar sections -> [ROWS, F, K] f32.

    Section 1 (rt0): [p, m, 2048 bytes] -> rows 0-127.
    Section 2 (merged rt1-3): [p, m, rt-1, 2048 bytes] -> rows 128-511.
    """
    b = raw.view(np.uint8)
    s0 = b[:, :2 * OW0].reshape(P, PLANES, F, 4)
    r0 = s0.transpose(0, 2, 3, 1).reshape(P, F, K)
    s1 = b[:, 2 * OW0:].reshape(P, PLANES, NRT - 1, F, 4)
    r1 = s1.transpose(2, 0, 3, 4, 1).reshape(ROWS - P, F, K)
    return np.concatenate([r0, r1], axis=0).astype(np.float32)


def kernel(x: np.ndarray) -> np.ndarray:
    from concourse.bass_utils import run_bass_kernel_spmd

    x = np.asarray(x, dtype=np.float32)
    assert x.shape == (ROWS_TOTAL, F), x.shape
    nc = _get_nc()
    in_maps = [
        {"xin": pack_shard(x[i * ROWS:(i + 1) * ROWS])} for i in range(N_CORES)
    ]
    res = run_bass_kernel_spmd(nc, in_maps, list(range(N_CORES)))
    parts = [unpack_shard(res.results[i]["out"]) for i in range(N_CORES)]
    return np.concatenate(parts, axis=0)


# revision 9
# speedup vs baseline: 1.0370x; 1.0360x over previous
"""Trainium2 Bass kernel: float32 -> 32-channel bit-plane encoding.

For input x [4096, 512] f32, produces out [4096, 512, 32] f32 where
out[b, f, 0] = (x[b,f] < 0) and out[b, f, 1+j] = bit (30-j) of
bitcast_int32(|x[b,f]|), MSB first.

Wire-format design: every output element is exactly 0.0 or 1.0, so the
device computes and stores each of the 67M output elements as a uint8
{0,1}; the host applies a value-preserving widening cast to f32.  This
cuts device HBM write traffic 4x (8MB/core instead of 32MB/core), which
is the binding roofline (per-NeuronCore HBM/fabric bandwidth ~430 GB/s
observed).

Host-side repack makes the device compute uniform:
  i' = (bitcast_u32(x) & 0x7FFFFFFF) | ((x < 0) << 31)
stored as a big-endian byte stream, viewed as uint16 pairs.  Then output
channel k of feature f equals bit (7 - k%8) of stream byte 4f + k//8.

Device compute (VectorE), one fused tensor_scalar op per bit plane:
  plane_m = (x_u16 >> (7-m)) & 0x0101     m = 0..7
Each uint16 element yields TWO planar output bytes; the dense step-1
16-bit single-src pattern hits the DVE 4x perf mode (~4 elem/cycle).

Schedule (engine-retire critical path = vector end + last piece issue):
- row tile 0: planes 0-1 column-split so compute starts on the first
  128KB input piece (~1.2us earlier); planes 2-7 full width.
- row tiles 1-3 fused into FD=3072 ops (amortizes per-op overhead).
- last plane split in two halves whose output pieces issue concurrently
  on the two HWDGE rings (SyncE / ScalarE), minimizing the tail.
Vector busy ~9.5us, fully under the output-DMA shadow.

The planes land in HBM planar per SBUF partition; the host interleaves
planes/tiles into [rows, F, 32] during the f32 cast.

Sharded row-wise over 8 NeuronCores (512 rows each, 4 row tiles of 128).
"""

import sys

if "/opt/trn_rl_repo" not in sys.path:
    sys.path.insert(0, "/opt/trn_rl_repo")

import numpy as np

import concourse.bass as bass
import concourse.mybir as mybir

P = 128           # SBUF partitions
F = 512           # features per row
K = 32            # output channels per feature
N_CORES = 8
ROWS_TOTAL = 4096
ROWS = ROWS_TOTAL // N_CORES   # rows per core (512)
NRT = ROWS // P                # row tiles per core (4)
W16 = F * 2                    # uint16 words per row (1024)
H16 = W16 // 2                 # half row (512)
PLANES = 8                     # bit planes per byte
MW = (NRT - 1) * W16           # merged row-tile width (3072 u16)
OW0 = PLANES * W16             # rt0 output section (8192 u16/partition)
OWM = PLANES * MW              # merged output section (24576 u16/partition)
OCOLS = OW0 + OWM              # output dram columns per partition (32768)


def build_nc() -> bass.Bass:
    nc = bass.Bass("TRN2", target_bir_lowering=False, debug=False)
    u16 = mybir.dt.uint16

    xin = nc.declare_dram_parameter("xin", [ROWS, W16], u16, isOutput=False)
    out = nc.declare_dram_parameter("out", [P, OCOLS], u16, isOutput=True)
    xin_ap, out_ap = xin.ap(), out.ap()

    shift_and = (mybir.AluOpType.logical_shift_right,
                 mybir.AluOpType.bitwise_and)

    # vector ops: (kind, m, lo, hi, wait) -- kind 0 = rt0 (xt0/ot0),
    # 1 = merged rt1-3 (xtm/otm); [lo,hi) u16 cols within the plane;
    # wait = list of (sem_idx, count) to wait before the op.
    vops = [
        (0, 0, 0, H16, [(0, 16)]),        # plane 0 first half: in0a
        (0, 0, H16, W16, [(0, 32)]),      # plane 0 second half: in0b
        (0, 1, 0, H16, []),
        (0, 1, H16, W16, []),
    ]
    vops += [(0, m, 0, W16, []) for m in range(2, PLANES)]
    vops += [(1, 0, 0, MW, [(1, 16), (2, 16), (3, 16)])]
    vops += [(1, m, 0, MW, []) for m in range(1, PLANES - 1)]
    vops += [(1, PLANES - 1, 0, MW // 2, []),
             (1, PLANES - 1, MW // 2, MW, [])]
    # out pieces: (engine 0=sync/1=scalar, dram col lo, hi, ts_count)
    pieces = [
        (0, 0, 4 * W16, 6),                     # rt0 planes 0-3 (1MB)
        (1, 4 * W16, 8 * W16, 10),              # rt0 planes 4-7 (1MB)
    ]
    for m in range(PLANES - 1):                 # merged planes (0.75MB)
        pieces.append((m % 2, OW0 + m * MW, OW0 + (m + 1) * MW, 11 + m))
    b7 = OW0 + (PLANES - 1) * MW
    pieces.append((0, b7, b7 + MW // 2, 18))    # plane 7 halves (0.375MB)
    pieces.append((1, b7 + MW // 2, b7 + MW, 19))

    from contextlib import ExitStack
    with ExitStack() as ctx:
        xt0 = ctx.enter_context(nc.sbuf_tensor("xt0", [P, W16], u16))
        xtm = ctx.enter_context(nc.sbuf_tensor("xtm", [P, MW], u16))
        ot0 = ctx.enter_context(nc.sbuf_tensor("ot0", [P, OW0], u16))
        otm = ctx.enter_context(nc.sbuf_tensor("otm", [P, OWM], u16))

        in_sem = [ctx.enter_context(nc.semaphore(f"in_sem{b}"))
                  for b in range(NRT)]
        ts_sem = ctx.enter_context(nc.semaphore("ts_sem"))
        od_sem = ctx.enter_context(nc.semaphore("od_sem"))

        ctx.enter_context(nc.Block())
        block = nc.cur_block

        @block.vector
        def _(vec: bass.BassEngine):
            for (kind, m, lo, hi, waits) in vops:
                for (si, cnt) in waits:
                    vec.wait_ge(in_sem[si], cnt)
                xt, ot, w = (xt0, ot0, W16) if kind == 0 else (xtm, otm, MW)
                vec.tensor_scalar(
                    ot[:, m * w + lo:m * w + hi],
                    xt[:, lo:hi],
                    7 - m,
                    0x0101,
                    *shift_and,
                ).then_inc(ts_sem)

        def piece_dma(eng, lo, hi, n):
            eng.wait_ge(ts_sem, n)
            src = ot0[:, lo:hi] if hi <= OW0 else otm[:, lo - OW0:hi - OW0]
            eng.dma_start(out_ap[:, lo:hi], src).then_inc(od_sem, 16)

        @block.scalar
        def _(sc: bass.BassEngine):
            # in0 split in halves for early vector start; in1 rides SyncE
            sc.dma_start(xt0[:, 0:H16],
                         xin_ap[0:P, 0:H16]).then_inc(in_sem[0], 16)
            sc.dma_start(xt0[:, H16:W16],
                         xin_ap[0:P, H16:W16]).then_inc(in_sem[0], 16)
            for rt in (2, 3):
                sc.dma_start(
                    xtm[:, (rt - 1) * W16:rt * W16],
                    xin_ap[rt * P:(rt + 1) * P, :],
                ).then_inc(in_sem[rt], 16)
            for (eng, lo, hi, n) in pieces:
                if eng == 1:
                    piece_dma(sc, lo, hi, n)

        @block.sync
        def _(sp: bass.BassEngine):
            sp.dma_start(
                xtm[:, 0:W16], xin_ap[P:2 * P, :]).then_inc(in_sem[1], 16)
            for (eng, lo, hi, n) in pieces:
                if eng == 0:
                    piece_dma(sp, lo, hi, n)

    return nc


_NC_CACHE = None


def _get_nc():
    global _NC_CACHE
    if _NC_CACHE is None:
        _NC_CACHE = build_nc()
    return _NC_CACHE


def pack_shard(x_shard: np.ndarray) -> np.ndarray:
    """[ROWS, F] f32 -> [ROWS, W16] uint16: sign-normalized bitcast words
    as a big-endian byte stream, viewed as little-endian uint16 pairs."""
    x_shard = np.ascontiguousarray(x_shard)
    xi = (x_shard.view(np.uint32) & np.uint32(0x7FFFFFFF)) | \
        ((x_shard < 0).astype(np.uint32) << np.uint32(31))
    return xi.byteswap().view(np.uint16)


def unpack_shard(raw: np.ndarray) -> np.ndarray:
    """[P, OCOLS] uint16 planar sections -> [ROWS, F, K] f32.

    Section 1 (rt0): [p, m, 2048 bytes] -> rows 0-127.
    Section 2 (merged rt1-3): [p, m, rt-1, 2048 bytes] -> rows 128-511.
    """
    b = raw.view(np.uint8)
    s0 = b[:, :2 * OW0].reshape(P, PLANES, F, 4)
    r0 = s0.transpose(0, 2, 3, 1).reshape(P, F, K)
    s1 = b[:, 2 * OW0:].reshape(P, PLANES, NRT - 1, F, 4)
    r1 = s1.transpose(2, 0, 3, 4, 1).reshape(ROWS - P, F, K)
    return np.concatenate([r0, r1], axis=0).astype(np.float32)


def kernel(x: np.ndarray) -> np.ndarray:
    from concourse.bass_utils import run_bass_kernel_spmd

    x = np.asarray(x, dtype=np.float32)
    assert x.shape == (ROWS_TOTAL, F), x.shape
    nc = _get_nc()
    in_maps = [
        {"xin": pack_shard(x[i * ROWS:(i + 1) * ROWS])} for i in range(N_CORES)
    ]
    res = run_bass_kernel_spmd(nc, in_maps, list(range(N_CORES)))
    parts = [unpack_shard(res.results[i]["out"]) for i in range(N_CORES)]
    return np.concatenate(parts, axis=0)


# revision 10
# speedup vs baseline: 1.0842x; 1.0455x over previous
"""Trainium2 Bass kernel: float32 -> 32-channel bit-plane encoding.

For input x [4096, 512] f32, produces out [4096, 512, 32] f32 where
out[b, f, 0] = (x[b,f] < 0) and out[b, f, 1+j] = bit (30-j) of
bitcast_int32(|x[b,f]|), MSB first.

Wire-format design: every output element is exactly 0.0 or 1.0, so the
device computes and stores each of the 67M output elements as a uint8
{0,1}; the host applies a value-preserving widening cast to f32.  This
cuts device HBM write traffic 4x (8MB/core instead of 32MB/core), which
is the binding roofline (per-NeuronCore HBM/fabric bandwidth ~430 GB/s
observed).

Host-side repack makes the device compute uniform:
  i' = (bitcast_u32(x) & 0x7FFFFFFF) | ((x < 0) << 31)
stored as a big-endian byte stream, viewed as uint16 pairs.  Then output
channel k of feature f equals bit (7 - k%8) of stream byte 4f + k//8.

Device compute (VectorE), one fused tensor_scalar op per bit plane:
  plane_m = (x_u16 >> (7-m)) & 0x0101     m = 0..7
Each uint16 element yields TWO planar output bytes; the dense step-1
16-bit single-src pattern hits the DVE 4x perf mode (~4 elem/cycle).

Measured critical path = preamble (7.3us, fixed) -> first input receipt
(~4us) -> vector stream (~9.7us) -> last output-piece issue -> teardown.
Schedule accordingly: row tile 0 loads first via SyncE (earliest-ready
engine); row tiles 1-3 are fused into FD=3072 ops (amortizes per-op
overhead); the final plane is split in halves whose pieces issue on
whichever HWDGE ring is free, and piece issue (~1.3us/MB of sequencer
time) is balanced across the SyncE and ScalarE rings.

The planes land in HBM planar per SBUF partition; the host interleaves
planes/tiles into [rows, F, 32] during the f32 cast.

Sharded row-wise over 8 NeuronCores (512 rows each, 4 row tiles of 128).
"""

import sys

if "/opt/trn_rl_repo" not in sys.path:
    sys.path.insert(0, "/opt/trn_rl_repo")

import numpy as np

import concourse.bass as bass
import concourse.mybir as mybir

P = 128           # SBUF partitions
F = 512           # features per row
K = 32            # output channels per feature
N_CORES = 8
ROWS_TOTAL = 4096
ROWS = ROWS_TOTAL // N_CORES   # rows per core (512)
NRT = ROWS // P                # row tiles per core (4)
W16 = F * 2                    # uint16 words per row (1024)
PLANES = 8                     # bit planes per byte
MW = (NRT - 1) * W16           # merged row-tile width (3072 u16)
OW0 = PLANES * W16             # rt0 output section (8192 u16/partition)
OWM = PLANES * MW              # merged output section (24576 u16/partition)
OCOLS = OW0 + OWM              # output dram columns per partition (32768)


def build_nc() -> bass.Bass:
    nc = bass.Bass("TRN2", target_bir_lowering=False, debug=False)
    u16 = mybir.dt.uint16

    xin = nc.declare_dram_parameter("xin", [ROWS, W16], u16, isOutput=False)
    out = nc.declare_dram_parameter("out", [P, OCOLS], u16, isOutput=True)
    xin_ap, out_ap = xin.ap(), out.ap()

    shift_and = (mybir.AluOpType.logical_shift_right,
                 mybir.AluOpType.bitwise_and)

    # vector ops: (kind, m, lo, hi) -- kind 0 = rt0 (xt0/ot0), 1 = merged
    # rt1-3 (xtm/otm); [lo,hi) u16 cols within the plane.
    vops = [(0, m, 0, W16) for m in range(PLANES)]           # ts 1..8
    vops += [(1, m, 0, MW) for m in range(PLANES - 1)]       # ts 9..15
    vops += [(1, PLANES - 1, 0, MW // 2),                    # ts 16
             (1, PLANES - 1, MW // 2, MW)]                   # ts 17
    # out pieces: (engine 0=sync/1=scalar, dram col lo, hi, ts_count)
    b7 = OW0 + (PLANES - 1) * MW
    pieces = [
        (0, 0, 4 * W16, 4),                      # rt0 planes 0-3 (1MB)
        (0, 4 * W16, 8 * W16, 8),                # rt0 planes 4-7 (1MB)
        (0, OW0 + 0 * MW, OW0 + 1 * MW, 9),      # merged planes (0.75MB)
        (0, OW0 + 1 * MW, OW0 + 2 * MW, 10),
        (0, OW0 + 2 * MW, OW0 + 3 * MW, 11),
        (0, OW0 + 3 * MW, OW0 + 4 * MW, 12),
        (0, OW0 + 4 * MW, OW0 + 5 * MW, 13),
        (1, OW0 + 5 * MW, OW0 + 6 * MW, 14),
        (1, OW0 + 6 * MW, OW0 + 7 * MW, 15),
        (0, b7, b7 + MW // 2, 16),               # plane 7 halves
        (1, b7 + MW // 2, b7 + MW, 17),
    ]

    from contextlib import ExitStack
    with ExitStack() as ctx:
        xt0 = ctx.enter_context(nc.sbuf_tensor("xt0", [P, W16], u16))
        xtm = ctx.enter_context(nc.sbuf_tensor("xtm", [P, MW], u16))
        ot0 = ctx.enter_context(nc.sbuf_tensor("ot0", [P, OW0], u16))
        otm = ctx.enter_context(nc.sbuf_tensor("otm", [P, OWM], u16))

        in_sem = [ctx.enter_context(nc.semaphore(f"in_sem{b}"))
                  for b in range(NRT)]
        ts_sem = ctx.enter_context(nc.semaphore("ts_sem"))
        od_sem = ctx.enter_context(nc.semaphore("od_sem"))

        ctx.enter_context(nc.Block(no_gpsimd_drain=True))
        block = nc.cur_block

        @block.vector
        def _(vec: bass.BassEngine):
            for i, (kind, m, lo, hi) in enumerate(vops):
                if i == 0:
                    vec.wait_ge(in_sem[0], 16)
                elif i == PLANES:
                    for rt in range(1, NRT):
                        vec.wait_ge(in_sem[rt], 16)
                xt, ot, w = (xt0, ot0, W16) if kind == 0 else (xtm, otm, MW)
                vec.tensor_scalar(
                    ot[:, m * w + lo:m * w + hi],
                    xt[:, lo:hi],
                    7 - m,
                    0x0101,
                    *shift_and,
                ).then_inc(ts_sem)

        def piece_dma(eng, lo, hi, n):
            eng.wait_ge(ts_sem, n)
            src = ot0[:, lo:hi] if hi <= OW0 else otm[:, lo - OW0:hi - OW0]
            eng.dma_start(out_ap[:, lo:hi], src).then_inc(od_sem, 16)

        @block.sync
        def _(sp: bass.BassEngine):
            # rt0 input load first: SyncE exits the preamble earliest
            sp.dma_start(xt0[:], xin_ap[0:P, :]).then_inc(in_sem[0], 16)
            for (eng, lo, hi, n) in pieces:
                if eng == 0:
                    piece_dma(sp, lo, hi, n)

        @block.scalar
        def _(sc: bass.BassEngine):
            for rt in range(1, NRT):
                sc.dma_start(
                    xtm[:, (rt - 1) * W16:rt * W16],
                    xin_ap[rt * P:(rt + 1) * P, :],
                ).then_inc(in_sem[rt], 16)
            for (eng, lo, hi, n) in pieces:
                if eng == 1:
                    piece_dma(sc, lo, hi, n)

    return nc


_NC_CACHE = None


def _get_nc():
    global _NC_CACHE
    if _NC_CACHE is None:
        _NC_CACHE = build_nc()
    return _NC_CACHE


def pack_shard(x_shard: np.ndarray) -> np.ndarray:
    """[ROWS, F] f32 -> [ROWS, W16] uint16: sign-normalized bitcast words
    as a big-endian byte stream, viewed as little-endian uint16 pairs."""
    x_shard = np.ascontiguousarray(x_shard)
    xi = (x_shard.view(np.uint32) & np.uint32(0x7FFFFFFF)) | \
        ((x_shard < 0).astype(np.uint32) << np.uint32(31))
    return xi.byteswap().view(np.uint16)


def unpack_shard(raw: np.ndarray) -> np.ndarray:
    """[P, OCOLS] uint16 planar sections -> [ROWS, F, K] f32.

    Section 1 (rt0): [p, m, 2048 bytes] -> rows 0-127.
    Section 2 (merged rt1-3): [p, m, rt-1, 2048 bytes] -> rows 128-511.
    """
    b = raw.view(np.uint8)
    s0 = b[:, :2 * OW0].reshape(P, PLANES, F, 4)
    r0 = s0.transpose(0, 2, 3, 1).reshape(P, F, K)
    s1 = b[:, 2 * OW0:].reshape(P, PLANES, NRT - 1, F, 4)
    r1 = s1.transpose(2, 0, 3, 4, 1).reshape(ROWS - P, F, K)
    return np.concatenate([r0, r1], axis=0).astype(np.float32)


def kernel(x: np.ndarray) -> np.ndarray:
    from concourse.bass_utils import run_bass_kernel_spmd

    x = np.asarray(x, dtype=np.float32)
    assert x.shape == (ROWS_TOTAL, F), x.shape
    nc = _get_nc()
    in_maps = [
        {"xin": pack_shard(x[i * ROWS:(i + 1) * ROWS])} for i in range(N_CORES)
    ]
    res = run_bass_kernel_spmd(nc, in_maps, list(range(N_CORES)))
    parts = [unpack_shard(res.results[i]["out"]) for i in range(N_CORES)]
    return np.concatenate(parts, axis=0)


# revision 11
# speedup vs baseline: 1.1059x; 1.0201x over previous
"""Trainium2 Bass kernel: float32 -> 32-channel bit-plane encoding.

For input x [4096, 512] f32, produces out [4096, 512, 32] f32 where
out[b, f, 0] = (x[b,f] < 0) and out[b, f, 1+j] = bit (30-j) of
bitcast_int32(|x[b,f]|), MSB first.

Wire-format design: every output element is exactly 0.0 or 1.0, so the
device computes and stores each of the 67M output elements as a uint8
{0,1}; the host applies a value-preserving widening cast to f32.  This
cuts device HBM write traffic 4x (8MB/core instead of 32MB/core), which
is the binding roofline (per-NeuronCore HBM/fabric bandwidth ~430 GB/s
observed).

Host-side repack makes the device compute uniform:
  i' = (bitcast_u32(x) & 0x7FFFFFFF) | ((x < 0) << 31)
stored as a big-endian byte stream, viewed as uint16 pairs.  Then output
channel k of feature f equals bit (7 - k%8) of stream byte 4f + k//8.

Device compute (VectorE), one fused tensor_scalar op per bit plane:
  plane_m = (x_u16 >> (7-m)) & 0x0101     m = 0..7
Each uint16 element yields TWO planar output bytes; the dense step-1
16-bit single-src pattern hits the DVE 4x perf mode (~4 elem/cycle).

Measured critical path = preamble (7.3us, fixed) -> first input receipt
(~4us) -> vector stream (~9.7us) -> last output-piece issue -> teardown.
Schedule accordingly: row tile 0 loads first via SyncE (earliest-ready
engine); row tiles 1-3 are fused into FD=3072 ops (amortizes per-op
overhead); the final plane is split in halves whose pieces issue on
whichever HWDGE ring is free, and piece issue (~1.3us/MB of sequencer
time) is balanced across the SyncE and ScalarE rings.

The planes land in HBM planar per SBUF partition; the host interleaves
planes/tiles into [rows, F, 32] during the f32 cast.

Sharded row-wise over 8 NeuronCores (512 rows each, 4 row tiles of 128).
"""

import sys

if "/opt/trn_rl_repo" not in sys.path:
    sys.path.insert(0, "/opt/trn_rl_repo")

import numpy as np

import concourse.bass as bass
import concourse.mybir as mybir

P = 128           # SBUF partitions
F = 512           # features per row
K = 32            # output channels per feature
N_CORES = 8
ROWS_TOTAL = 4096
ROWS = ROWS_TOTAL // N_CORES   # rows per core (512)
NRT = ROWS // P                # row tiles per core (4)
W16 = F * 2                    # uint16 words per row (1024)
PLANES = 8                     # bit planes per byte
MW = (NRT - 1) * W16           # merged row-tile width (3072 u16)
OW0 = PLANES * W16             # rt0 output section (8192 u16/partition)
OWM = PLANES * MW              # merged output section (24576 u16/partition)
OCOLS = OW0 + OWM              # output dram columns per partition (32768)


def build_nc() -> bass.Bass:
    nc = bass.Bass("TRN2", target_bir_lowering=False, debug=False)
    u16 = mybir.dt.uint16

    xin = nc.declare_dram_parameter("xin", [ROWS, W16], u16, isOutput=False)
    out = nc.declare_dram_parameter("out", [P, OCOLS], u16, isOutput=True)
    xin_ap, out_ap = xin.ap(), out.ap()

    shift_and = (mybir.AluOpType.logical_shift_right,
                 mybir.AluOpType.bitwise_and)

    # vector ops: (kind, m, lo, hi) -- kind 0 = rt0 (xt0/ot0), 1 = merged
    # rt1-3 (xtm/otm); [lo,hi) u16 cols within the plane.
    vops = [(0, m, 0, W16) for m in range(PLANES)]           # ts 1..8
    vops += [(1, m, 0, MW) for m in range(PLANES - 1)]       # ts 9..15
    vops += [(1, PLANES - 1, 0, MW // 2),                    # ts 16
             (1, PLANES - 1, MW // 2, MW)]                   # ts 17
    # out pieces: (engine 0=sync/1=scalar, dram col lo, hi, ts_count)
    b7 = OW0 + (PLANES - 1) * MW
    pieces = [
        (0, 0, 4 * W16, 4),                      # rt0 planes 0-3 (1MB)
        (1, 4 * W16, 8 * W16, 8),                # rt0 planes 4-7 (1MB)
        (0, OW0 + 0 * MW, OW0 + 1 * MW, 9),      # merged planes (0.75MB)
        (1, OW0 + 1 * MW, OW0 + 2 * MW, 10),
        (0, OW0 + 2 * MW, OW0 + 3 * MW, 11),
        (1, OW0 + 3 * MW, OW0 + 4 * MW, 12),
        (0, OW0 + 4 * MW, OW0 + 5 * MW, 13),
        (1, OW0 + 5 * MW, OW0 + 6 * MW, 14),
        (0, OW0 + 6 * MW, OW0 + 7 * MW, 15),
        (1, b7, b7 + MW // 2, 16),               # plane 7 halves
        (0, b7 + MW // 2, b7 + MW, 17),
    ]

    from contextlib import ExitStack
    with ExitStack() as ctx:
        xt0 = ctx.enter_context(nc.sbuf_tensor("xt0", [P, W16], u16))
        xtm = ctx.enter_context(nc.sbuf_tensor("xtm", [P, MW], u16))
        ot0 = ctx.enter_context(nc.sbuf_tensor("ot0", [P, OW0], u16))
        otm = ctx.enter_context(nc.sbuf_tensor("otm", [P, OWM], u16))

        in_sem = [ctx.enter_context(nc.semaphore(f"in_sem{b}"))
                  for b in range(NRT)]
        ts_sem = ctx.enter_context(nc.semaphore("ts_sem"))
        od_sem = ctx.enter_context(nc.semaphore("od_sem"))

        ctx.enter_context(nc.Block(no_gpsimd_drain=True))
        block = nc.cur_block

        @block.vector
        def _(vec: bass.BassEngine):
            for i, (kind, m, lo, hi) in enumerate(vops):
                if i == 0:
                    vec.wait_ge(in_sem[0], 16)
                elif i == PLANES:
                    for rt in range(1, NRT):
                        vec.wait_ge(in_sem[rt], 16)
                xt, ot, w = (xt0, ot0, W16) if kind == 0 else (xtm, otm, MW)
                vec.tensor_scalar(
                    ot[:, m * w + lo:m * w + hi],
                    xt[:, lo:hi],
                    7 - m,
                    0x0101,
                    *shift_and,
                ).then_inc(ts_sem)

        def piece_dma(eng, lo, hi, n):
            eng.wait_ge(ts_sem, n)
            src = ot0[:, lo:hi] if hi <= OW0 else otm[:, lo - OW0:hi - OW0]
            eng.dma_start(out_ap[:, lo:hi], src).then_inc(od_sem, 16)

        @block.sync
        def _(sp: bass.BassEngine):
            # rt0 input load first: SyncE exits the preamble earliest
            sp.dma_start(xt0[:], xin_ap[0:P, :]).then_inc(in_sem[0], 16)
            for (eng, lo, hi, n) in pieces:
                if eng == 0:
                    piece_dma(sp, lo, hi, n)

        @block.scalar
        def _(sc: bass.BassEngine):
            for rt in range(1, NRT):
                sc.dma_start(
                    xtm[:, (rt - 1) * W16:rt * W16],
                    xin_ap[rt * P:(rt + 1) * P, :],
                ).then_inc(in_sem[rt], 16)
            for (eng, lo, hi, n) in pieces:
                if eng == 1:
                    piece_dma(sc, lo, hi, n)

    return nc


_NC_CACHE = None


def _get_nc():
    global _NC_CACHE
    if _NC_CACHE is None:
        _NC_CACHE = build_nc()
    return _NC_CACHE


def pack_shard(x_shard: np.ndarray) -> np.ndarray:
    """[ROWS, F] f32 -> [ROWS, W16] uint16: sign-normalized bitcast words
    as a big-endian byte stream, viewed as little-endian uint16 pairs."""
    x_shard = np.ascontiguousarray(x_shard)
    xi = (x_shard.view(np.uint32) & np.uint32(0x7FFFFFFF)) | \
        ((x_shard < 0).astype(np.uint32) << np.uint32(31))
    return xi.byteswap().view(np.uint16)


def unpack_shard(raw: np.ndarray) -> np.ndarray:
    """[P, OCOLS] uint16 planar sections -> [ROWS, F, K] f32.

    Section 1 (rt0): [p, m, 2048 bytes] -> rows 0-127.
    Section 2 (merged rt1-3): [p, m, rt-1, 2048 bytes] -> rows 128-511.
    """
    b = raw.view(np.uint8)
    s0 = b[:, :2 * OW0].reshape(P, PLANES, F, 4)
    r0 = s0.transpose(0, 2, 3, 1).reshape(P, F, K)
    s1 = b[:, 2 * OW0:].reshape(P, PLANES, NRT - 1, F, 4)
    r1 = s1.transpose(2, 0, 3, 4, 1).reshape(ROWS - P, F, K)
    return np.concatenate([r0, r1], axis=0).astype(np.float32)


def kernel(x: np.ndarray) -> np.ndarray:
    from concourse.bass_utils import run_bass_kernel_spmd

    x = np.asarray(x, dtype=np.float32)
    assert x.shape == (ROWS_TOTAL, F), x.shape
    nc = _get_nc()
    in_maps = [
        {"xin": pack_shard(x[i * ROWS:(i + 1) * ROWS])} for i in range(N_CORES)
    ]
    res = run_bass_kernel_spmd(nc, in_maps, list(range(N_CORES)))
    parts = [unpack_shard(res.results[i]["out"]) for i in range(N_CORES)]
    return np.concatenate(parts, axis=0)
